# revision 2
# baseline (speedup 1.0000x reference)
"""GRU (B=512, T=512, I=32, H=64) + linear head, data-parallel over 8 NeuronCores.

Per core (B_local=64), layout [hidden/gate on partitions, batch on free dim]:
  - x is PE-transposed on-chip into xT[i, (t,b)] tiles (32-partition groups).
  - Per step t, PSUM accumulates  a_rz = W_ih_rz.x_t (+) W_hh_rz.h + b_rz  via two
    matmuls (x-part prefetched one step ahead, bias via an all-ones row in the
    h tile, K=65).
  - r,z = sigmoid(a_rz) as two ACT ops (everything stays at partitions 0-63).
  - n = tanh(gx_n + b_ih_n + r*(gh_n + b_hh_n)); h' = z*h + (1-z)*n on DVE.
  - y_t = W_lin.h_t + b_lin as a per-step matmul into a 32-step PSUM bank,
    evacuated to SBUF every 32 steps and DMA'd out per 64-step chunk.
"""

import numpy as np
import concourse.bass as bass
import concourse.mybir as mybir
from concourse.tile import TileContext
from concourse.vector_clock import ScopedClock
from concourse.bass_utils import run_bass_kernel_spmd

B, T, I, O, H = 512, 512, 32, 16, 64
NCORES = 8
BL = B // NCORES            # 64 batch rows per core
S = 64                      # steps per x/y chunk
YB = 32                     # y steps batched per PSUM bank (32*16 = 512 fp32)
f32 = mybir.dt.float32
AF = mybir.ActivationFunctionType
ALU = mybir.AluOpType


class _TC(TileContext):
    """TileContext whose tail/body instructions never carry >2 sem waits.

    This walrus build enforces a hard 2-sync-wait-per-instruction limit;
    Tile's scheduler occasionally emits more (notably the kernel-tail drain
    and matmuls waiting on several DMA queues). Split the excess onto
    same-engine nops inserted immediately before the offending instruction.
    """

    def _drain_and_barrier(self, tick_clock, wait_clock):
        super()._drain_and_barrier(tick_clock, wait_clock)
        nc = self.nc
        for fn in nc.m.functions:
            for blk in fn.blocks:
                out = []
                for inst in blk.instructions:
                    si = getattr(inst, "sync_info", None)
                    waits = list(si.on_wait) if si and si.on_wait else []
                    limit = 1
                    if len(waits) > limit:
                        si.on_wait = waits[-limit:]
                        extra = waits[:-limit]
                        for k in range(len(extra)):
                            eng = nc.engines[inst.engine]
                            nop = eng.nop(nofuse=True)
                            cur = nc.cur_bb.bb.instructions
                            assert cur and cur[-1] is nop.ins
                            cur.pop()
                            nop.ins.sync_info = mybir.SyncInfo(
                                on_wait=[extra[k]], on_update=[])
                            out.append(nop.ins)
                    out.append(inst)
                blk.instructions[:] = out


def build_bass(t_steps=T, s_chunk=S, io_steps=None):
    n_chunk = t_steps // s_chunk
    io_steps = io_steps or t_steps
    nio = io_steps // s_chunk
    nc = bass.Bass("TRN2", target_bir_lowering=False, debug=False,
                   num_devices=NCORES)
    x_d = nc.dram_tensor("x", [BL, io_steps * I], f32, kind="ExternalInput")
    wrz_d = nc.dram_tensor("w_rz", [H + 1, 2 * H], f32, kind="ExternalInput")
    wn_d = nc.dram_tensor("w_n", [H + 1, H], f32, kind="ExternalInput")
    wxrz_d = nc.dram_tensor("w_xrz", [4 * I, 2 * H], f32, kind="ExternalInput")
    wxn_d = nc.dram_tensor("w_xn", [4 * I, H], f32, kind="ExternalInput")
    wlin_d = nc.dram_tensor("w_lin", [H + 1, O], f32, kind="ExternalInput")
    bn_d = nc.dram_tensor("b_n", [H, 1], f32, kind="ExternalInput")
    id_d = nc.dram_tensor("ident", [BL, BL], f32, kind="ExternalInput")
    y_d = nc.dram_tensor("y", [BL, io_steps * O], f32, kind="ExternalOutput")

    gpw = s_chunk // 4          # transpose groups per chunk
    yb = min(YB, s_chunk)       # y steps per PSUM bank

    with _TC(nc) as tc:
        with (
            tc.tile_pool(name="const", bufs=1) as cpool,
            tc.tile_pool(name="state", bufs=1) as spool,
            tc.tile_pool(name="work", bufs=2) as wpool,
            tc.tile_pool(name="psum", bufs=1, space="PSUM") as ppool,
        ):
            w_rz = cpool.tile([H + 1, 2 * H], f32)
            nc.sync.dma_start(w_rz[:, :], wrz_d[:, :])
            w_n = cpool.tile([H + 1, H], f32)
            nc.sync.dma_start(w_n[:, :], wn_d[:, :])
            w_xrz = cpool.tile([4 * I, 2 * H], f32)
            nc.sync.dma_start(w_xrz[:, :], wxrz_d[:, :])
            w_xn = cpool.tile([4 * I, H], f32)
            nc.sync.dma_start(w_xn[:, :], wxn_d[:, :])
            w_lin = cpool.tile([H + 1, O], f32)
            nc.sync.dma_start(w_lin[:, :], wlin_d[:, :])
            b_n = cpool.tile([H, 1], f32)
            nc.sync.dma_start(b_n[:, :], bn_d[:, :])
            ident = cpool.tile([BL, BL], f32)
            nc.sync.dma_start(ident[:, :], id_d[:, :])

            hh = spool.tile([H + 1, 2 * BL], f32)          # h slots + ones row
            xT = spool.tile([128, 2 * gpw * BL], f32)      # transposed x ring
            xs = spool.tile([BL, 2 * s_chunk * I], f32)    # raw x ring
            ysb = spool.tile([BL, 2 * s_chunk * O], f32)   # y staging ring

            nc.vector.memset(hh[0:H, :], 0.0)
            nc.vector.memset(hh[H:H + 1, :], 1.0)

            def produce_group(c, g):
                ci = c % nio
                if g == 0:
                    nc.sync.dma_start(
                        xs[:, (c % 2) * s_chunk * I:((c % 2) + 1) * s_chunk * I],
                        x_d[:, ci * s_chunk * I:(ci + 1) * s_chunk * I])
                tp = ppool.tile([128, BL], f32, tag="tp", bufs=1,
                                name=f"tp_{c}_{g}")
                nc.tensor.transpose(
                    tp[:, :],
                    xs[:, (c % 2) * s_chunk * I + g * 128:
                       (c % 2) * s_chunk * I + (g + 1) * 128],
                    ident[:, :])
                col = (c % 2) * gpw * BL + g * BL
                nc.scalar.activation(xT[:, col:col + BL], tp[:, :], AF.Copy)

            def xt_slice(t):
                c, tl = divmod(t, s_chunk)
                g, p = divmod(tl, 4)
                col = (c % 2) * gpw * BL + g * BL
                return xT[p * 32:(p + 1) * 32, col:col + BL]

            def new_rz_ps(t):
                p = (t % s_chunk) % 4
                ps = ppool.tile([2 * H, BL], f32, tag="rz", bufs=2,
                                name=f"rz_ps_{t}")
                nc.tensor.matmul(ps[:, :], w_xrz[p * I:(p + 1) * I, :],
                                 xt_slice(t), start=True, stop=False,
                                 tile_position=(p * I, 0))
                return ps

            def new_gxn_ps(t):
                p = (t % s_chunk) % 4
                ps = ppool.tile([H, BL], f32, tag="gxn", bufs=2,
                                name=f"gxn_ps_{t}")
                nc.tensor.matmul(ps[:, :], w_xn[p * I:(p + 1) * I, :],
                                 xt_slice(t), start=True, stop=True,
                                 tile_position=(p * I, 0))
                return ps

            # prologue: chunk 0 producer + step 0 x-side matmuls
            for g in range(gpw):
                produce_group(0, g)
            rz_ps = new_rz_ps(0)
            gxn_ps = new_gxn_ps(0)
            y_ps = ppool.tile([BL, yb * O], f32, tag="y", bufs=2, name="y_ps_0")

            rz_next = None
            gxn_next = None
            for t in range(t_steps):
                c, tl = divmod(t, s_chunk)
                rd = ((t - 1) % 2) * BL
                wr = (t % 2) * BL
                h_prev = hh[0:H, rd:rd + BL]
                h_prev_aug = hh[0:H + 1, rd:rd + BL]

                # --- PE: critical recurrent matmuls
                nc.tensor.matmul(rz_ps[:, :], w_rz[:, :], h_prev_aug,
                                 start=False, stop=True)
                n_ps = ppool.tile([H, BL], f32, tag="n", bufs=1,
                                  name=f"n_ps_{t}")
                nc.tensor.matmul(n_ps[:, :], w_n[:, :], h_prev_aug,
                                 start=True, stop=True)

                # --- PE: y projection for step t-1 (h_{t-1} is ready)
                if t >= 1:
                    yi = ((t - 1) % yb) * O
                    nc.tensor.matmul(y_ps[:, yi:yi + O], h_prev_aug,
                                     w_lin[:, :], start=True, stop=True)
                if t % yb == 0 and t >= yb:
                    blk = t // yb - 1                     # completed y block
                    cb = (blk * yb) // s_chunk            # its chunk
                    dst = (cb % 2) * s_chunk * O + (blk * yb % s_chunk) * O
                    nc.scalar.activation(ysb[:, dst:dst + yb * O], y_ps[:, :],
                                         AF.Copy)
                    y_ps = ppool.tile([BL, yb * O], f32, tag="y", bufs=2,
                                      name=f"y_ps_{t}")
                if t % s_chunk == 0 and t >= s_chunk:
                    cb = c - 1
                    cbi = cb % nio
                    src = (cb % 2) * s_chunk * O
                    nc.sync.dma_start(
                        y_d[:, cbi * s_chunk * O:(cbi + 1) * s_chunk * O],
                        ysb[:, src:src + s_chunk * O])

                # --- PE: producer for chunk c+1, spread across the chunk
                if tl % 4 == 0 and c + 1 < n_chunk:
                    produce_group(c + 1, tl // 4)

                # --- PE: x-side prefetch for step t+1
                if t + 1 < t_steps:
                    rz_next = new_rz_ps(t + 1)
                    gxn_next = new_gxn_ps(t + 1)

                # --- ACT: gates
                r_sb = wpool.tile([H, BL], f32, tag="r", name=f"r_{t}")
                nc.scalar.activation(r_sb[:, :], rz_ps[0:H, :], AF.Sigmoid)
                z_sb = wpool.tile([H, BL], f32, tag="z", name=f"z_{t}")
                nc.scalar.activation(z_sb[:, :], rz_ps[H:2 * H, :], AF.Sigmoid)

                # --- DVE: n pre-activation
                t1 = wpool.tile([H, BL], f32, tag="t1", name=f"t1_{t}")
                nc.vector.tensor_tensor(t1[:, :], n_ps[:, :], r_sb[:, :],
                                        ALU.mult)
                t2 = wpool.tile([H, BL], f32, tag="t2", name=f"t2_{t}")
                nc.vector.tensor_tensor(t2[:, :], t1[:, :], gxn_ps[:, :],
                                        ALU.add)

                # --- ACT: n = tanh(t2 + b_ih_n)
                n_sb = wpool.tile([H, BL], f32, tag="n_sb", name=f"n_{t}")
                nc.scalar.activation(n_sb[:, :], t2[:, :], AF.Tanh,
                                     bias=b_n[:, 0:1])

                # --- DVE: blend h' = z*h + (1-z)*n
                zc = wpool.tile([H, BL], f32, tag="zc", name=f"zc_{t}")
                nc.vector.tensor_scalar(zc[:, :], z_sb[:, :], -1.0, 1.0,
                                        ALU.mult, ALU.add)
                q = wpool.tile([H, BL], f32, tag="q", name=f"q_{t}")
                nc.vector.tensor_tensor(q[:, :], z_sb[:, :], h_prev, ALU.mult)
                w_sb = wpool.tile([H, BL], f32, tag="w", name=f"w_{t}")
                nc.vector.tensor_tensor(w_sb[:, :], zc[:, :], n_sb[:, :],
                                        ALU.mult)
                nc.vector.tensor_tensor(hh[0:H, wr:wr + BL], q[:, :],
                                        w_sb[:, :], ALU.add)

                rz_ps = rz_next
                gxn_ps = gxn_next

            # epilogue: last y projection + final staging + final chunk DMA
            rdl = ((t_steps - 1) % 2) * BL
            yi = ((t_steps - 1) % yb) * O
            nc.tensor.matmul(y_ps[:, yi:yi + O], hh[0:H + 1, rdl:rdl + BL],
                             w_lin[:, :], start=True, stop=True)
            blk = t_steps // yb - 1
            cb = (blk * yb) // s_chunk
            dst = (cb % 2) * s_chunk * O + (blk * yb % s_chunk) * O
            nc.scalar.activation(ysb[:, dst:dst + yb * O], y_ps[:, :], AF.Copy)
            src = (cb % 2) * s_chunk * O
            cbi = cb % nio
            nc.sync.dma_start(
                y_d[:, cbi * s_chunk * O:(cbi + 1) * s_chunk * O],
                ysb[:, src:src + s_chunk * O])
    return nc


def prep_consts(W_ih, W_hh, b_ih, b_hh, W_lin, b_lin):
    W_ih = np.asarray(W_ih, np.float32)
    W_hh = np.asarray(W_hh, np.float32)
    b_ih = np.asarray(b_ih, np.float32)
    b_hh = np.asarray(b_hh, np.float32)
    W_lin = np.asarray(W_lin, np.float32)
    b_lin = np.asarray(b_lin, np.float32)
    return {
        "w_rz": np.ascontiguousarray(np.concatenate(
            [W_hh[0:2 * H].T, (b_ih[0:2 * H] + b_hh[0:2 * H])[None, :]], 0)),
        "w_n": np.ascontiguousarray(np.concatenate(
            [W_hh[2 * H:3 * H].T, b_hh[2 * H:3 * H][None, :]], 0)),
        "w_xrz": np.ascontiguousarray(np.tile(W_ih[0:2 * H].T, (4, 1))),
        "w_xn": np.ascontiguousarray(np.tile(W_ih[2 * H:3 * H].T, (4, 1))),
        "w_lin": np.ascontiguousarray(np.concatenate(
            [W_lin.T, b_lin[None, :]], 0)),
        "b_n": np.ascontiguousarray(b_ih[2 * H:3 * H].reshape(H, 1)),
        "ident": np.eye(BL, dtype=np.float32),
    }


_cached = {}


def build_in_maps(np_inputs):
    x = np.asarray(np_inputs["x"], np.float32)
    consts = prep_consts(np_inputs["W_ih"], np_inputs["W_hh"],
                         np_inputs["b_ih"], np_inputs["b_hh"],
                         np_inputs["W_lin"], np_inputs["b_lin"])
    in_maps = []
    for cid in range(NCORES):
        m = dict(consts)
        m["x"] = np.ascontiguousarray(
            x[cid * BL:(cid + 1) * BL].reshape(BL, T * I))
        in_maps.append(m)
    return in_maps


def kernel(x, W_ih, W_hh, b_ih, b_hh, W_lin, b_lin):
    if "nc" not in _cached:
        _cached["nc"] = build_bass()
    nc = _cached["nc"]
    in_maps = build_in_maps(dict(x=x, W_ih=W_ih, W_hh=W_hh, b_ih=b_ih,
                                 b_hh=b_hh, W_lin=W_lin, b_lin=b_lin))
    res = run_bass_kernel_spmd(nc, in_maps, core_ids=list(range(NCORES)))
    out = np.concatenate(
        [res.results[cid]["y"].reshape(BL, T, O) for cid in range(NCORES)], 0)
    return out

